# revision 1
# baseline (speedup 1.0000x reference)
"""Trainium2 Bass kernel for batched 3-D k-NN local-covariance trace.

Problem: pcd [B=8, N=4096, 3] -> per-point trace of the 3x3 covariance of its
k=5 nearest neighbors (self included), normalized by the per-batch max.

Sharding: data-parallel over batch — core b owns batch b (N=4096 points).

Per-core algorithm (all SBUF-resident after the initial load):
  * rank value r[i,j] = 2*x_i.x_j - |x_i|^2 - |x_j|^2 = -d2[i,j], computed as a
    single K=5 augmented matmul:  lhsT rows [2x,2y,2z,-sq,1], rhs rows
    [x,y,z,1,-sq].  Row-block of 128 queries x 8 chunks of 512 candidates.
  * top-5 neighbors per query via DVE max (top-8 values) + max_index
    (first-occurrence indices, ties resolve to the lowest index like
    jax.lax.top_k).
  * neighbor coordinate gather via gpsimd indirect_copy: the idxs tile is read
    wrapped per 16-partition core group in (slot-major, query-minor) order, so
    passing the max_index tile [:, :5] directly makes each group gather its own
    16 queries' neighbors from a table with coords on partitions 16g..16g+2.
  * stable centered trace per query (sum of squared deviations from the
    5-neighbor mean), components summed across partitions with a tiny matmul
    against a constant selection matrix E.
  * global max over the 4096 traces (gpsimd partition_all_reduce) -> scale by
    1/(max+1e-8) -> DMA out.
"""

import numpy as np
from contextlib import ExitStack

N = 4096
KNN = 5
P = 128          # queries per row block
NBLK = N // P    # 32 row blocks
CH = 512         # candidate chunk (one fp32 PSUM bank)
NCH = N // CH    # 8 chunks
G16 = 16         # partitions per gpsimd core group
NG = P // G16    # 8 groups per row block


def build_nc():
    import concourse.bass as bass
    import concourse.tile as tile
    from concourse import bacc, mybir
    from concourse import bass_isa

    dt = mybir.dt
    f32 = dt.float32
    Alu = mybir.AluOpType
    Axis = mybir.AxisListType

    nc = bacc.Bacc("TRN2", target_bir_lowering=False, debug=False)
    pcd_d = nc.dram_tensor("pcd", [N, 3], f32, kind="ExternalInput")
    out_d = nc.dram_tensor("out", [N], f32, kind="ExternalOutput")
    pcd_t = pcd_d.ap().rearrange("n d -> d n")      # [3, N] strided view

    with tile.TileContext(nc) as tc, ExitStack() as ctx:
        const = ctx.enter_context(tc.tile_pool(name="const", bufs=1))
        mpool = ctx.enter_context(tc.tile_pool(name="mval", bufs=2))
        small = ctx.enter_context(tc.tile_pool(name="small", bufs=3))
        psum = ctx.enter_context(tc.tile_pool(name="psum", bufs=6, space="PSUM"))
        psacc = ctx.enter_context(tc.tile_pool(name="psacc", bufs=1, space="PSUM"))

        # ---- one-time setup -------------------------------------------------
        xr = const.tile([5, N], f32)         # rhs rows [x,y,z,1,-sq]
        xl = const.tile([5, N], f32)         # lhsT rows [2x,2y,2z,-sq,1]
        tbl = const.tile([P, N], f32)        # gather table: coords on p%16<3

        # coords into xr/xl rows 0-2 straight from DRAM (parallel queues);
        # per-row DMAs so transfers overlap instead of queueing on one engine
        for d, eng in enumerate((nc.sync, nc.scalar, nc.gpsimd)):
            eng.dma_start(xr[d:d + 1, :], pcd_t[d:d + 1, :])
        for d, eng in enumerate((nc.scalar, nc.gpsimd, nc.sync)):
            eng.dma_start(xl[d:d + 1, :], pcd_t[d:d + 1, :])
        nc.gpsimd.memset(tbl[:], 0.0)
        nc.scalar.dma_start(tbl[0:3, :], pcd_t)

        nc.scalar.mul(xl[0:3, :], xl[0:3, :], 2.0)

        s3 = const.tile([3, N], f32)         # squared coords
        nc.vector.tensor_mul(s3[:], xr[0:3, :], xr[0:3, :])

        ones3 = const.tile([3, 1], f32)
        nc.vector.memset(ones3[:], 1.0)
        ones1 = const.tile([1, N], f32)
        nc.vector.memset(ones1[:], 1.0)

        sq_neg = const.tile([1, N], f32)
        for c in range(NCH):
            sl = slice(c * CH, (c + 1) * CH)
            sq_ps = psum.tile([1, CH], f32, tag="mm")
            nc.tensor.matmul(sq_ps[:], ones3[:], s3[:, sl], start=True, stop=True)
            nc.scalar.mul(sq_neg[0:1, sl], sq_ps[:], -1.0)

        # assemble remaining rows via DMA (arbitrary partition offsets),
        # spread across engine queues so they run concurrently
        nc.sync.dma_start(xr[3:4, :], ones1[:])
        nc.gpsimd.dma_start(xr[4:5, :], sq_neg[:])
        nc.scalar.dma_start(xl[3:4, :], sq_neg[:])
        nc.sync.dma_start(xl[4:5, :], ones1[:])

        # replicate coords to every 16-partition group of tbl
        engs = (nc.sync, nc.scalar, nc.gpsimd, nc.sync,
                nc.scalar, nc.gpsimd, nc.sync)
        for g in range(1, NG):
            engs[g - 1].dma_start(tbl[G16 * g:G16 * g + 3, :], tbl[0:3, :])

        # E[p, g] = 1 iff p//16 == g and p%16 < 3  (component-sum selector)
        esel = const.tile([P, NG], f32)
        nc.vector.memset(esel[:], 0.0)
        for g in range(NG):
            nc.sync.dma_start(esel[G16 * g:G16 * g + 3, g:g + 1], ones3[:])

        trace_ps = psacc.tile([G16, NG * NBLK], f32)   # [16, 256], one bank

        # ---- main loop over row blocks -------------------------------------
        for r in range(NBLK):
            lhsT = xl[:, r * P:(r + 1) * P]
            mval = mpool.tile([P, N], f32)
            for c in range(NCH):
                sl = slice(c * CH, (c + 1) * CH)
                ps = psum.tile([P, CH], f32, tag="mm")
                nc.tensor.matmul(ps[:], lhsT, xr[:, sl], start=True, stop=True)
                nc.scalar.copy(mval[:, sl], ps[:])

            v8 = small.tile([P, 8], f32, tag="v8")
            nc.vector.max(v8[:], mval[:])
            idx8 = small.tile([P, 8], dt.uint16, tag="idx8")
            nc.vector.max_index(idx8[:], v8[:], mval[:])

            # gather: group g gathers, for its 16 queries, slot-major:
            # gath[p, s*16+q16] = tbl[p, idx8[16*(p//16)+q16, s]]
            gath = small.tile([P, KNN * G16], f32, tag="gath")
            nc.gpsimd.indirect_copy(gath[:], tbl[:], idx8[:, :KNN], True)

            gv = gath[:].rearrange("p (s q) -> p q s", s=KNN, q=G16)
            ssum = small.tile([P, G16], f32, tag="ssum")
            nc.vector.tensor_reduce(ssum[:], gv, axis=Axis.X, op=Alu.add)
            mean = small.tile([P, G16], f32, tag="mean")
            nc.scalar.mul(mean[:], ssum[:], 1.0 / KNN)

            cent = small.tile([P, G16, KNN], f32, tag="cent")
            nc.gpsimd.tensor_sub(cent[:], gv,
                                 mean[:].unsqueeze(2).broadcast_to([P, G16, KNN]))
            nc.gpsimd.tensor_mul(cent[:], cent[:], cent[:])
            tt = small.tile([P, G16], f32, tag="tt")
            nc.vector.tensor_reduce(tt[:], cent[:], axis=Axis.X, op=Alu.add)

            nc.tensor.matmul(trace_ps[:, r * NG:(r + 1) * NG], tt[:], esel[:],
                             start=True, stop=True)

        # ---- normalize + store ---------------------------------------------
        tr_sb = const.tile([G16, NG * NBLK], f32)
        nc.scalar.copy(tr_sb[:], trace_ps[:])
        gmax = const.tile([G16, 1], f32)
        nc.vector.tensor_reduce(gmax[:], tr_sb[:], axis=Axis.X, op=Alu.max)
        gmax_all = const.tile([G16, 1], f32)
        nc.gpsimd.partition_all_reduce(gmax_all[:], gmax[:], channels=G16,
                                       reduce_op=bass_isa.ReduceOp.max)
        denom = const.tile([G16, 1], f32)
        nc.vector.tensor_scalar_add(denom[:], gmax_all[:], 1e-8)
        rec = const.tile([G16, 1], f32)
        nc.vector.reciprocal(rec[:], denom[:])
        outv = const.tile([G16, NG * NBLK], f32)
        nc.vector.tensor_scalar_mul(outv[:], tr_sb[:], rec[:])

        nc.sync.dma_start(
            out_d.ap().rearrange("(r g q) -> q (r g)", r=NBLK, g=NG, q=G16),
            outv[:],
        )

    nc.compile()
    return nc


_NC_CACHE = {}


def kernel(pcd, k):
    pcd = np.asarray(pcd)
    k = int(np.asarray(k))
    assert k == KNN, f"kernel hardcodes k={KNN}, got {k}"
    B, n, d = pcd.shape
    assert (n, d) == (N, 3), f"kernel hardcodes N={N}, got {(n, d)}"

    from concourse.bass_utils import run_bass_kernel_spmd

    if "nc" not in _NC_CACHE:
        _NC_CACHE["nc"] = build_nc()
    nc = _NC_CACHE["nc"]

    in_maps = [{"pcd": np.ascontiguousarray(pcd[b], dtype=np.float32)}
               for b in range(B)]
    res = run_bass_kernel_spmd(nc, in_maps, list(range(B)))
    out = np.stack([res.results[b]["out"] for b in range(B)], axis=0)
    return out.astype(np.float32, copy=False)


if __name__ == "__main__":
    x = np.random.randn(8, N, 3).astype(np.float32)
    y = kernel(x, 5)
    print(y.shape, y.dtype, y[:2, :4])



# revision 13
# speedup vs baseline: 1.4844x; 1.4844x over previous
"""Trainium2 Bass kernel for batched 3-D k-NN local-covariance trace.

Problem: pcd [B=8, N=4096, 3] -> per-point trace of the 3x3 covariance of its
k=5 nearest neighbors (self included), normalized by the per-batch max.

Sharding: data-parallel over batch — core b owns batch b (N=4096 points).

Per-core algorithm (all SBUF-resident after the initial load):
  * rank value r[i,j] = 2*x_i.y_j - |y_j|^2  (the -|x_i|^2 term is constant
    per row so it cannot change the per-query ordering; r = -d2 + const_i).
    Computed as a K=11 bf16 matmul with an error-free hi/lo split so the
    product is accurate to ~2^-17 relative (hi*hi + hi*lo + lo*hi terms):
      lhsT rows: [xh x3, xh x3, xlo x3, 1, 1]
      rhs  rows: [2yh x3, 2ylo x3, 2yh x3, -sqh, -sqlo]
    bf16 streams 1 column/cycle on the PE (vs 4 for fp32) and enables FWL
    fast weight loads.  Row-block of 128 queries x 1024-candidate chunks.
  * top-5 neighbors per query via DVE max (top-8 values) + max_index
    (first-occurrence indices, ties resolve to the lowest index like
    jax.lax.top_k).
  * neighbor coordinate gather via gpsimd indirect_copy: the idxs tile is read
    wrapped per 16-partition core group in (slot-major, query-minor) order, so
    passing the max_index tile [:, :5] directly makes each group gather its own
    16 queries' neighbors from a table with coords on partitions 16g..16g+2.
  * stable centered trace per query (sum of squared deviations from the
    5-neighbor mean), components summed across partitions with a tiny matmul
    against a constant selection matrix E.
  * global max over the 4096 traces (gpsimd partition_all_reduce) -> scale by
    1/(max+1e-8) -> DMA out.
"""

import numpy as np
from contextlib import ExitStack

N = 4096
KNN = 5
P = 128          # queries per row block
NBLK = N // P    # 32 row blocks
CH = 512         # candidate chunk (one fp32 PSUM bank)
NCH = N // CH    # 4 chunks
G16 = 16         # partitions per gpsimd core group
NG = P // G16    # 8 groups per row block
KA = 11          # augmented contraction rows (hi/lo split)


def build_nc():
    import concourse.bass as bass
    import concourse.tile as tile
    from concourse import bacc, mybir
    from concourse import bass_isa

    dt = mybir.dt
    f32 = dt.float32
    bf16 = dt.bfloat16
    Alu = mybir.AluOpType
    Axis = mybir.AxisListType

    nc = bacc.Bacc("TRN2", target_bir_lowering=False, debug=False)
    pcd_d = nc.dram_tensor("pcd", [N, 3], f32, kind="ExternalInput")
    out_d = nc.dram_tensor("out", [N], f32, kind="ExternalOutput")
    pcd_t = pcd_d.ap().rearrange("n d -> d n")      # [3, N] strided view

    with tile.TileContext(nc) as tc, ExitStack() as ctx:
        const = ctx.enter_context(tc.tile_pool(name="const", bufs=1))
        mpool = ctx.enter_context(tc.tile_pool(name="mval", bufs=2))
        small = ctx.enter_context(tc.tile_pool(name="small", bufs=3))
        psum = ctx.enter_context(tc.tile_pool(name="psum", bufs=6, space="PSUM"))
        psacc = ctx.enter_context(tc.tile_pool(name="psacc", bufs=1, space="PSUM"))

        # ---- one-time setup -------------------------------------------------
        stage_cm = tc.tile_pool(name="stage", bufs=1)
        stage = stage_cm.__enter__()
        sqps_cm = tc.tile_pool(name="sqps", bufs=1, space="PSUM")
        sqps = sqps_cm.__enter__()
        x3 = stage.tile([3, N], f32)         # coords [x,y,z] fp32
        tbl = const.tile([P, N], f32)        # gather table: coords on p%16<3

        for d, eng in enumerate((nc.sync, nc.scalar, nc.gpsimd)):
            eng.dma_start(x3[d:d + 1, :], pcd_t[d:d + 1, :])
        nc.gpsimd.memset(tbl[:], 0.0)
        nc.scalar.dma_start(tbl[0:3, :], pcd_t)

        s3 = stage.tile([3, N], f32)         # squared coords
        nc.vector.tensor_mul(s3[:], x3[:], x3[:])

        ones3 = stage.tile([3, 1], f32)
        nc.vector.memset(ones3[:], 1.0)

        # sq_neg = -|y|^2 (fp32, exact contraction via fp32 matmul chunks)
        sq_neg = stage.tile([1, N], f32)
        for c in range(8):
            sl = slice(c * 512, (c + 1) * 512)
            sq_ps = sqps.tile([1, 512], f32, tag="sq")
            nc.tensor.matmul(sq_ps[:], ones3[:], s3[:, sl], start=True, stop=True)
            nc.scalar.mul(sq_neg[0:1, sl], sq_ps[:], -1.0)

        # hi/lo split staging (coords on partitions 0-2, sq on partition 0)
        st_h = stage.tile([3, N], bf16)      # bf16(x)
        nc.scalar.copy(st_h[:], x3[:])
        st_2h = stage.tile([3, N], bf16)     # bf16(2x) == 2*bf16(x)
        nc.scalar.mul(st_2h[:], x3[:], 2.0)
        st_hf = stage.tile([3, N], f32)
        nc.vector.tensor_copy(st_hf[:], st_h[:])
        st_lo = stage.tile([3, N], f32)      # x - fp32(bf16(x)), exact
        nc.vector.tensor_sub(st_lo[:], x3[:], st_hf[:])
        st_lob = stage.tile([3, N], bf16)
        nc.scalar.copy(st_lob[:], st_lo[:])
        st_2lob = stage.tile([3, N], bf16)
        nc.scalar.mul(st_2lob[:], st_lo[:], 2.0)

        st_sqh = stage.tile([1, N], bf16)    # bf16(-sq)
        nc.scalar.copy(st_sqh[:], sq_neg[:])
        st_sqhf = st_hf[0:1, :]              # reuse coord staging space
        nc.vector.tensor_copy(st_sqhf, st_sqh[:])
        st_sqlf = st_lo[0:1, :]
        nc.vector.tensor_sub(st_sqlf, sq_neg[:], st_sqhf)
        st_sqlb = stage.tile([1, N], bf16)
        nc.scalar.copy(st_sqlb[:], st_sqlf)

        # pack lhsT rows [xh,xh,xlo,1,1] and rhs rows [2yh,2ylo,2yh,-sqh,-sqlo]
        ones_b = stage.tile([2, N], bf16)
        nc.vector.memset(ones_b[:], 1.0)
        xl = const.tile([KA, N], bf16)
        xr = const.tile([KA, N], bf16)
        nc.sync.dma_start(xl[0:3, :], st_h[:])
        nc.scalar.dma_start(xl[3:6, :], st_h[:])
        nc.gpsimd.dma_start(xl[6:9, :], st_lob[:])
        nc.sync.dma_start(xl[9:11, :], ones_b[:])
        nc.sync.dma_start(xr[0:3, :], st_2h[:])
        nc.scalar.dma_start(xr[3:6, :], st_2lob[:])
        nc.gpsimd.dma_start(xr[6:9, :], st_2h[:])
        nc.sync.dma_start(xr[9:10, :], st_sqh[:])
        nc.scalar.dma_start(xr[10:11, :], st_sqlb[:])

        # replicate coords to every 16-partition group of tbl
        engs = (nc.sync, nc.scalar, nc.gpsimd, nc.sync,
                nc.scalar, nc.gpsimd, nc.sync)
        for g in range(1, NG):
            engs[g - 1].dma_start(tbl[G16 * g:G16 * g + 3, :], tbl[0:3, :])

        # E[p, g] = 1 iff p//16 == g and p%16 < 3  (component-sum selector)
        esel = const.tile([P, NG], f32)
        nc.vector.memset(esel[:], 0.0)
        for g in range(NG):
            nc.sync.dma_start(esel[G16 * g:G16 * g + 3, g:g + 1], ones3[:])

        stage_cm.__exit__(None, None, None)
        sqps_cm.__exit__(None, None, None)

        trace_ps = psacc.tile([G16, NG * NBLK], f32)   # [16, 256], one bank

        # ---- main loop over row blocks -------------------------------------
        for r in range(NBLK):
            lhsT = xl[:, r * P:(r + 1) * P]
            mval = mpool.tile([P, N], f32)
            for c in range(NCH):
                sl = slice(c * CH, (c + 1) * CH)
                ps = psum.tile([P, CH], f32, tag="mm")
                nc.tensor.matmul(ps[:], lhsT, xr[:, sl], start=True, stop=True)
                nc.scalar.copy(mval[:, sl], ps[:])

            v8 = small.tile([P, 8], f32, tag="v8")
            nc.vector.max(v8[:], mval[:])
            idx8 = small.tile([P, 8], dt.uint16, tag="idx8")
            nc.vector.max_index(idx8[:], v8[:], mval[:])

            # gather: group g gathers, for its 16 queries, slot-major:
            # gath[p, s*16+q16] = tbl[p, idx8[16*(p//16)+q16, s]]
            gath = small.tile([P, KNN * G16], f32, tag="gath")
            nc.gpsimd.indirect_copy(gath[:], tbl[:], idx8[:, :KNN], True)

            gv = gath[:].rearrange("p (s q) -> p q s", s=KNN, q=G16)
            ssum = small.tile([P, G16], f32, tag="ssum")
            nc.vector.tensor_reduce(ssum[:], gv, axis=Axis.X, op=Alu.add)
            mean = small.tile([P, G16], f32, tag="mean")
            nc.scalar.mul(mean[:], ssum[:], 1.0 / KNN)

            cent = small.tile([P, G16, KNN], f32, tag="cent")
            nc.gpsimd.tensor_sub(cent[:], gv,
                                 mean[:].unsqueeze(2).broadcast_to([P, G16, KNN]))
            nc.gpsimd.tensor_mul(cent[:], cent[:], cent[:])
            tt = small.tile([P, G16], f32, tag="tt")
            nc.vector.tensor_reduce(tt[:], cent[:], axis=Axis.X, op=Alu.add)

            nc.tensor.matmul(trace_ps[:, r * NG:(r + 1) * NG], tt[:], esel[:],
                             start=True, stop=True)

        # ---- normalize + store ---------------------------------------------
        tr_sb = const.tile([G16, NG * NBLK], f32)
        nc.scalar.copy(tr_sb[:], trace_ps[:])
        gmax = const.tile([G16, 1], f32)
        nc.vector.tensor_reduce(gmax[:], tr_sb[:], axis=Axis.X, op=Alu.max)
        gmax_all = const.tile([G16, 1], f32)
        nc.gpsimd.partition_all_reduce(gmax_all[:], gmax[:], channels=G16,
                                       reduce_op=bass_isa.ReduceOp.max)
        denom = const.tile([G16, 1], f32)
        nc.vector.tensor_scalar_add(denom[:], gmax_all[:], 1e-8)
        rec = const.tile([G16, 1], f32)
        nc.vector.reciprocal(rec[:], denom[:])
        outv = const.tile([G16, NG * NBLK], f32)
        nc.vector.tensor_scalar_mul(outv[:], tr_sb[:], rec[:])

        nc.sync.dma_start(
            out_d.ap().rearrange("(r g q) -> q (r g)", r=NBLK, g=NG, q=G16),
            outv[:],
        )

    nc.compile()
    return nc


_NC_CACHE = {}


def kernel(pcd, k):
    pcd = np.asarray(pcd)
    k = int(np.asarray(k))
    assert k == KNN, f"kernel hardcodes k={KNN}, got {k}"
    B, n, d = pcd.shape
    assert (n, d) == (N, 3), f"kernel hardcodes N={N}, got {(n, d)}"

    from concourse.bass_utils import run_bass_kernel_spmd

    if "nc" not in _NC_CACHE:
        _NC_CACHE["nc"] = build_nc()
    nc = _NC_CACHE["nc"]

    in_maps = [{"pcd": np.ascontiguousarray(pcd[b], dtype=np.float32)}
               for b in range(B)]
    res = run_bass_kernel_spmd(nc, in_maps, list(range(B)))
    out = np.stack([res.results[b]["out"] for b in range(B)], axis=0)
    return out.astype(np.float32, copy=False)


if __name__ == "__main__":
    x = np.random.randn(8, N, 3).astype(np.float32)
    y = kernel(x, 5)
    print(y.shape, y.dtype, y[:2, :4])


# revision 21
# speedup vs baseline: 1.9933x; 1.3429x over previous
"""Trainium2 Bass kernel for batched 3-D k-NN local-covariance trace.

Problem: pcd [B=8, N=4096, 3] -> per-point trace of the 3x3 covariance of its
k=5 nearest neighbors (self included), normalized by the per-batch max.

Sharding: data-parallel over batch — core b owns batch b (N=4096 points).

Per-core algorithm (all SBUF-resident after the initial load):
  * rank value r[i,j] = 2*x_i.y_j - |y_j|^2  (the -|x_i|^2 term is constant
    per row so it cannot change the per-query ordering; r = -d2 + const_i).
    Computed as a K=11 bf16 matmul with an error-free hi/lo split so the
    product is accurate to ~2^-17 relative (hi*hi + hi*lo + lo*hi terms):
      lhsT rows: [xh x3, xh x3, xlo x3, 1, 1]
      rhs  rows: [2yh x3, 2ylo x3, 2yh x3, -sqh, -sqlo]
    bf16 streams 1 column/cycle on the PE (vs 4 for fp32) and enables FWL
    fast weight loads.  Row-block of 128 queries x 512-candidate chunks.
  * setup avoids the 4-byte-strided pcd loads (they explode into ~33k DMA
    packets): pcd is DMA'd once contiguously as [128, 96], the hi/lo split
    runs elementwise in that layout on all 128 lanes, and the [*, N] rows
    for the matmul / gather table are produced by DVE 32x32 block
    transposes + contiguous flatten DMAs.  This stores points in a fixed
    permutation n' (k = 1024b + 32j + pb  <->  point 1024b + 32pb + j);
    every tensor uses the same permutation, so only the final output DMA
    needs to invert it (the data has no exact distance ties, so tie-break
    order between permutations cannot change the neighbor sets).
  * top-5 neighbors per query via DVE max (top-8 values) + max_index
    (first-occurrence indices).
  * neighbor coordinate gather via gpsimd indirect_copy: the idxs tile is read
    wrapped per 16-partition core group in (slot-major, query-minor) order, so
    passing the max_index tile [:, :5] directly makes each group gather its own
    16 queries' neighbors from a table with coords on partitions 16g..16g+2.
  * stable centered trace per query (sum of squared deviations from the
    5-neighbor mean), components summed across partitions with a tiny matmul
    against a constant selection matrix E.
  * global max over the 4096 traces (gpsimd partition_all_reduce) -> scale by
    1/(max+1e-8) -> two-queue DMA out.
"""

import numpy as np
from contextlib import ExitStack

N = 4096
KNN = 5
P = 128          # queries per row block
NBLK = N // P    # 32 row blocks
CH = 512         # candidate chunk (one fp32 PSUM bank)
NCH = N // CH    # 8 chunks
G16 = 16         # partitions per gpsimd core group
NG = P // G16    # 8 groups per row block
KA = 11          # augmented contraction rows (hi/lo split)


def build_nc():
    import concourse.bass as bass
    import concourse.tile as tile
    from concourse import bacc, mybir
    from concourse import bass_isa

    dt = mybir.dt
    f32 = dt.float32
    bf16 = dt.bfloat16
    Alu = mybir.AluOpType
    Axis = mybir.AxisListType

    nc = bacc.Bacc("TRN2", target_bir_lowering=False, debug=False)
    pcd_d = nc.dram_tensor("pcd", [N, 3], f32, kind="ExternalInput")
    out_d = nc.dram_tensor("out", [N], f32, kind="ExternalOutput")

    with tile.TileContext(nc) as tc, ExitStack() as ctx:
        const = ctx.enter_context(tc.tile_pool(name="const", bufs=1))
        stage = ctx.enter_context(tc.tile_pool(name="stage", bufs=1))
        mpool = ctx.enter_context(tc.tile_pool(name="mval", bufs=2))
        small = ctx.enter_context(tc.tile_pool(name="small", bufs=3))
        psum = ctx.enter_context(tc.tile_pool(name="psum", bufs=6, space="PSUM"))
        psacc = ctx.enter_context(tc.tile_pool(name="psacc", bufs=2, space="PSUM"))

        # ---- one-time setup -------------------------------------------------
        # contiguous load: partition p holds points 32p..32p+31 (96 floats)
        pw = stage.tile([P, 3 * 32], f32)
        nc.sync.dma_start(pw[:], pcd_d.ap().rearrange("(p j) d -> p (j d)", p=P))

        # hi/lo split, elementwise in the wide layout (all 128 lanes)
        ph = stage.tile([P, 3 * 32], bf16)       # bf16(x)
        nc.scalar.copy(ph[:], pw[:])
        p2h = stage.tile([P, 3 * 32], bf16)      # bf16(2x) == 2*bf16(x)
        nc.scalar.mul(p2h[:], pw[:], 2.0)
        phf = stage.tile([P, 3 * 32], f32)
        nc.vector.tensor_copy(phf[:], ph[:])
        plo = stage.tile([P, 3 * 32], f32)       # x - fp32(bf16(x)), exact
        nc.vector.tensor_sub(plo[:], pw[:], phf[:])
        plob = stage.tile([P, 3 * 32], bf16)
        nc.scalar.copy(plob[:], plo[:])
        p2lob = stage.tile([P, 3 * 32], bf16)
        nc.scalar.mul(p2lob[:], plo[:], 2.0)

        # -|y|^2 per point, then its own hi/lo split
        s3p = stage.tile([P, 3 * 32], f32)
        nc.vector.tensor_mul(s3p[:], pw[:], pw[:])
        sqn = stage.tile([P, 32], f32)
        nc.vector.tensor_reduce(sqn[:], s3p[:].rearrange("p (j d) -> p j d", d=3),
                                axis=Axis.X, op=Alu.add, negate=True)
        snh = stage.tile([P, 32], bf16)
        nc.scalar.copy(snh[:], sqn[:])
        snhf = stage.tile([P, 32], f32)
        nc.vector.tensor_copy(snhf[:], snh[:])
        snlo = stage.tile([P, 32], f32)
        nc.vector.tensor_sub(snlo[:], sqn[:], snhf[:])
        snlob = stage.tile([P, 32], bf16)
        nc.scalar.copy(snlob[:], snlo[:])

        # 32x32 block transposes; flattening (partition-major) then yields the
        # permuted point order n' shared by every [*, N] tensor below.
        def coord_view(t, d):
            return t[:].rearrange("p (j d) -> p d j", d=3)[:, d, :]

        tbl = const.tile([P, N], f32)        # gather table: coords on p%16<3
        xl = const.tile([KA, N], bf16)
        xr = const.tile([KA, N], bf16)

        tp = [stage.tile([P, 32], f32, name=f"tp{d}") for d in range(3)]
        th = [stage.tile([P, 32], bf16, name=f"th{d}") for d in range(3)]
        t2h = [stage.tile([P, 32], bf16, name=f"t2h{d}") for d in range(3)]
        tlo = [stage.tile([P, 32], bf16, name=f"tlo{d}") for d in range(3)]
        t2lo = [stage.tile([P, 32], bf16, name=f"t2lo{d}") for d in range(3)]
        tsh = stage.tile([P, 32], bf16)
        tslo = stage.tile([P, 32], bf16)
        for d in range(3):
            nc.vector.transpose(tp[d][:], coord_view(pw, d))
            nc.vector.transpose(th[d][:], coord_view(ph, d))
            nc.vector.transpose(t2h[d][:], coord_view(p2h, d))
            nc.vector.transpose(tlo[d][:], coord_view(plob, d))
            nc.vector.transpose(t2lo[d][:], coord_view(p2lob, d))
        nc.vector.transpose(tsh[:], snh[:])
        nc.vector.transpose(tslo[:], snlob[:])

        # flatten DMAs (contiguous 128B/64B runs) into the row tiles
        qs = (nc.sync, nc.scalar, nc.gpsimd)
        nc.gpsimd.memset(tbl[:], 0.0)
        for d in range(3):
            qs[d].dma_start(tbl[d:d + 1, :], tp[d][:])
            qs[d].dma_start(xl[d:d + 1, :], th[d][:])
            qs[(d + 1) % 3].dma_start(xl[6 + d:7 + d, :], tlo[d][:])
            qs[(d + 1) % 3].dma_start(xr[d:d + 1, :], t2h[d][:])
            qs[(d + 2) % 3].dma_start(xr[3 + d:4 + d, :], t2lo[d][:])
        nc.sync.dma_start(xr[9:10, :], tsh[:])
        nc.scalar.dma_start(xr[10:11, :], tslo[:])
        # duplicated row groups via one contiguous copy each
        nc.gpsimd.dma_start(xl[3:6, :], xl[0:3, :])
        nc.sync.dma_start(xr[6:9, :], xr[0:3, :])
        ones_b = stage.tile([2, N], bf16)
        nc.vector.memset(ones_b[:], 1.0)
        nc.scalar.dma_start(xl[9:11, :], ones_b[:])

        # replicate coords to every 16-partition group of tbl
        engs = (nc.sync, nc.scalar, nc.gpsimd, nc.sync,
                nc.scalar, nc.gpsimd, nc.sync)
        for g in range(1, NG):
            engs[g - 1].dma_start(tbl[G16 * g:G16 * g + 3, :], tbl[0:3, :])

        # E[p, g] = 1 iff p//16 == g and p%16 < 3  (component-sum selector)
        ones3 = const.tile([3, 1], f32)
        nc.vector.memset(ones3[:], 1.0)
        esel = const.tile([P, NG], f32)
        nc.vector.memset(esel[:], 0.0)
        for j in range(NG):
            g = 2 * (j & 3) + (j >> 2)
            nc.sync.dma_start(esel[G16 * g:G16 * g + 3, j:j + 1], ones3[:])

        tr_sb = const.tile([G16, NG * NBLK], f32)
        # free layout of tr_sb: f = 64b + 32gl + 4rb + gh for row block
        # r = 8b + rb and group g = 2gh + gl  ->  DRAM block b is contiguous
        tr_view = tr_sb[:].rearrange("q (b gl rb gh) -> q b gl rb gh",
                                     b=4, gl=2, rb=8, gh=4)

        # ---- main loop over row blocks -------------------------------------
        for r in range(NBLK):
            lhsT = xl[:, r * P:(r + 1) * P]
            mval = mpool.tile([P, N], f32)
            for c in range(NCH):
                sl = slice(c * CH, (c + 1) * CH)
                ps = psum.tile([P, CH], f32, tag="mm")
                nc.tensor.matmul(ps[:], lhsT, xr[:, sl], start=True, stop=True)
                nc.scalar.copy(mval[:, sl], ps[:])

            v8 = small.tile([P, 8], f32, tag="v8")
            nc.vector.max(v8[:], mval[:])
            idx8 = small.tile([P, 8], dt.uint16, tag="idx8")
            nc.vector.max_index(idx8[:], v8[:], mval[:])

            # gather: group g gathers, for its 16 queries, slot-major:
            # gath[p, s*16+q16] = tbl[p, idx8[16*(p//16)+q16, s]]
            gath = small.tile([P, KNN * G16], f32, tag="gath")
            nc.gpsimd.indirect_copy(gath[:], tbl[:], idx8[:, :KNN], True)

            gv = gath[:].rearrange("p (s q) -> p q s", s=KNN, q=G16)
            ssum = small.tile([P, G16], f32, tag="ssum")
            nc.vector.tensor_reduce(ssum[:], gv, axis=Axis.X, op=Alu.add)
            mean = small.tile([P, G16], f32, tag="mean")
            nc.scalar.mul(mean[:], ssum[:], 1.0 / KNN)

            cent = small.tile([P, G16, KNN], f32, tag="cent")
            nc.gpsimd.tensor_sub(cent[:], gv,
                                 mean[:].unsqueeze(2).broadcast_to([P, G16, KNN]))
            nc.gpsimd.tensor_mul(cent[:], cent[:], cent[:])
            tt = small.tile([P, G16], f32, tag="tt")
            nc.vector.tensor_reduce(tt[:], cent[:], axis=Axis.X, op=Alu.add)

            ps_tr = psacc.tile([G16, NG], f32, tag="tr")
            nc.tensor.matmul(ps_tr[:], tt[:], esel[:], start=True, stop=True)
            nc.scalar.copy(tr_view[:, r // 8, :, r % 8, :],
                           ps_tr[:].rearrange("q (gl gh) -> q gl gh", gl=2))

        # ---- normalize + store ---------------------------------------------
        gmax = const.tile([G16, 1], f32)
        nc.vector.tensor_reduce(gmax[:], tr_sb[:], axis=Axis.X, op=Alu.max)
        gmax_all = const.tile([G16, 1], f32)
        nc.gpsimd.partition_all_reduce(gmax_all[:], gmax[:], channels=G16,
                                       reduce_op=bass_isa.ReduceOp.max)
        denom = const.tile([G16, 1], f32)
        nc.vector.tensor_scalar_add(denom[:], gmax_all[:], 1e-8)
        rec = const.tile([G16, 1], f32)
        nc.vector.reciprocal(rec[:], denom[:])
        outv = const.tile([G16, NG * NBLK], f32)
        nc.vector.tensor_scalar_mul(outv[:], tr_sb[:], rec[:])

        # invert the permutation: query at wrapped slot (q16, r*8+g) with
        # r = 8b+rb, g = 2gh+gl is point n = 1024b + 512gl + 32q16 + 4rb + gh
        # per-b DMA: n = 1024b + 512gl + 32q + (4rb+gh); partition q must be
        # the outermost SBUF dim, innermost runs are 32 contiguous elements
        ov = outv[:].rearrange("q (b gl rbgh) -> b q gl rbgh",
                               b=4, gl=2, rbgh=32)
        od = out_d.ap().rearrange("(b gl q rbgh) -> b q gl rbgh",
                                  b=4, gl=2, q=G16, rbgh=32)
        qs2 = (nc.sync, nc.scalar, nc.gpsimd, nc.sync)
        for b in range(4):
            qs2[b].dma_start(od[b], ov[b])

    nc.compile()
    return nc


_NC_CACHE = {}


def kernel(pcd, k):
    pcd = np.asarray(pcd)
    k = int(np.asarray(k))
    assert k == KNN, f"kernel hardcodes k={KNN}, got {k}"
    B, n, d = pcd.shape
    assert (n, d) == (N, 3), f"kernel hardcodes N={N}, got {(n, d)}"

    from concourse.bass_utils import run_bass_kernel_spmd

    if "nc" not in _NC_CACHE:
        _NC_CACHE["nc"] = build_nc()
    nc = _NC_CACHE["nc"]

    in_maps = [{"pcd": np.ascontiguousarray(pcd[b], dtype=np.float32)}
               for b in range(B)]
    res = run_bass_kernel_spmd(nc, in_maps, list(range(B)))
    out = np.stack([res.results[b]["out"] for b in range(B)], axis=0)
    return out.astype(np.float32, copy=False)


if __name__ == "__main__":
    x = np.random.randn(8, N, 3).astype(np.float32)
    y = kernel(x, 5)
    print(y.shape, y.dtype, y[:2, :4])


# revision 24
# speedup vs baseline: 2.0068x; 1.0067x over previous
"""Trainium2 Bass kernel for batched 3-D k-NN local-covariance trace.

Problem: pcd [B=8, N=4096, 3] -> per-point trace of the 3x3 covariance of its
k=5 nearest neighbors (self included), normalized by the per-batch max.

Sharding: data-parallel over batch — core b owns batch b (N=4096 points).

Per-core algorithm (all SBUF-resident after the initial load):
  * rank value r[i,j] = 2*x_i.y_j - |y_j|^2  (the -|x_i|^2 term is constant
    per row so it cannot change the per-query ordering; r = -d2 + const_i).
    Computed as a K=11 bf16 matmul with an error-free hi/lo split so the
    product is accurate to ~2^-17 relative (hi*hi + hi*lo + lo*hi terms):
      lhsT rows: [xh x3, xh x3, xlo x3, 1, 1]
      rhs  rows: [2yh x3, 2ylo x3, 2yh x3, -sqh, -sqlo]
    bf16 streams 1 column/cycle on the PE (vs 4 for fp32) and enables FWL
    fast weight loads.  Row-block of 128 queries x 512-candidate chunks.
  * setup avoids the 4-byte-strided pcd loads (they explode into ~33k DMA
    packets): pcd is DMA'd once contiguously as [128, 96], the hi/lo split
    runs elementwise in that layout on all 128 lanes, and the [*, N] rows
    for the matmul / gather table are produced by DVE 32x32 block
    transposes + contiguous flatten DMAs.  This stores points in a fixed
    permutation n' (k = 1024b + 32j + pb  <->  point 1024b + 32pb + j);
    every tensor uses the same permutation, so only the final output DMA
    needs to invert it (the data has no exact distance ties, so tie-break
    order between permutations cannot change the neighbor sets).
  * top-5 neighbors per query via DVE max (top-8 values) + max_index
    (first-occurrence indices).
  * neighbor coordinate gather via gpsimd indirect_copy: the idxs tile is read
    wrapped per 16-partition core group in (slot-major, query-minor) order, so
    passing the max_index tile [:, :5] directly makes each group gather its own
    16 queries' neighbors from a table with coords on partitions 16g..16g+2.
  * stable centered trace per query (sum of squared deviations from the
    5-neighbor mean), components summed across partitions with a tiny matmul
    against a constant selection matrix E.
  * global max over the 4096 traces (gpsimd partition_all_reduce) -> scale by
    1/(max+1e-8) -> two-queue DMA out.
"""

import numpy as np
from contextlib import ExitStack

N = 4096
KNN = 5
P = 128          # queries per row block
NBLK = N // P    # 32 row blocks
CH = 512         # candidate chunk (one fp32 PSUM bank)
NCH = N // CH    # 8 chunks
G16 = 16         # partitions per gpsimd core group
NG = P // G16    # 8 groups per row block
KA = 11          # augmented contraction rows (hi/lo split)


def build_nc():
    import concourse.bass as bass
    import concourse.tile as tile
    from concourse import bacc, mybir
    from concourse import bass_isa

    dt = mybir.dt
    f32 = dt.float32
    bf16 = dt.bfloat16
    Alu = mybir.AluOpType
    Axis = mybir.AxisListType

    nc = bacc.Bacc("TRN2", target_bir_lowering=False, debug=False)
    pcd_d = nc.dram_tensor("pcd", [N, 3], f32, kind="ExternalInput")
    out_d = nc.dram_tensor("out", [N], f32, kind="ExternalOutput")

    with tile.TileContext(nc) as tc, ExitStack() as ctx:
        const = ctx.enter_context(tc.tile_pool(name="const", bufs=1))
        stage = ctx.enter_context(tc.tile_pool(name="stage", bufs=1))
        mpool = ctx.enter_context(tc.tile_pool(name="mval", bufs=3))
        small = ctx.enter_context(tc.tile_pool(name="small", bufs=3))
        psum = ctx.enter_context(tc.tile_pool(name="psum", bufs=6, space="PSUM"))
        psacc = ctx.enter_context(tc.tile_pool(name="psacc", bufs=2, space="PSUM"))

        # ---- one-time setup -------------------------------------------------
        # contiguous load: partition p holds points 32p..32p+31 (96 floats)
        pw = stage.tile([P, 3 * 32], f32)
        nc.sync.dma_start(pw[:], pcd_d.ap().rearrange("(p j) d -> p (j d)", p=P))

        # hi/lo split, elementwise in the wide layout (all 128 lanes)
        ph = stage.tile([P, 3 * 32], bf16)       # bf16(x)
        nc.scalar.copy(ph[:], pw[:])
        p2h = stage.tile([P, 3 * 32], bf16)      # bf16(2x) == 2*bf16(x)
        nc.scalar.mul(p2h[:], pw[:], 2.0)
        phf = stage.tile([P, 3 * 32], f32)
        nc.vector.tensor_copy(phf[:], ph[:])
        plo = stage.tile([P, 3 * 32], f32)       # x - fp32(bf16(x)), exact
        nc.vector.tensor_sub(plo[:], pw[:], phf[:])
        plob = stage.tile([P, 3 * 32], bf16)
        nc.scalar.copy(plob[:], plo[:])
        p2lob = stage.tile([P, 3 * 32], bf16)
        nc.scalar.mul(p2lob[:], plo[:], 2.0)

        # -|y|^2 per point, then its own hi/lo split
        s3p = stage.tile([P, 3 * 32], f32)
        nc.vector.tensor_mul(s3p[:], pw[:], pw[:])
        sqn = stage.tile([P, 32], f32)
        nc.vector.tensor_reduce(sqn[:], s3p[:].rearrange("p (j d) -> p j d", d=3),
                                axis=Axis.X, op=Alu.add, negate=True)
        snh = stage.tile([P, 32], bf16)
        nc.scalar.copy(snh[:], sqn[:])
        snhf = stage.tile([P, 32], f32)
        nc.vector.tensor_copy(snhf[:], snh[:])
        snlo = stage.tile([P, 32], f32)
        nc.vector.tensor_sub(snlo[:], sqn[:], snhf[:])
        snlob = stage.tile([P, 32], bf16)
        nc.scalar.copy(snlob[:], snlo[:])

        # 32x32 block transposes; flattening (partition-major) then yields the
        # permuted point order n' shared by every [*, N] tensor below.
        def coord_view(t, d):
            return t[:].rearrange("p (j d) -> p d j", d=3)[:, d, :]

        tbl = const.tile([P, N], f32)        # gather table: coords on p%16<3
        xl = const.tile([KA, N], bf16)
        xr = const.tile([KA, N], bf16)

        tp = [stage.tile([P, 32], f32, name=f"tp{d}") for d in range(3)]
        th = [stage.tile([P, 32], bf16, name=f"th{d}") for d in range(3)]
        t2h = [stage.tile([P, 32], bf16, name=f"t2h{d}") for d in range(3)]
        tlo = [stage.tile([P, 32], bf16, name=f"tlo{d}") for d in range(3)]
        t2lo = [stage.tile([P, 32], bf16, name=f"t2lo{d}") for d in range(3)]
        tsh = stage.tile([P, 32], bf16)
        tslo = stage.tile([P, 32], bf16)
        for d in range(3):
            nc.vector.transpose(tp[d][:], coord_view(pw, d))
            nc.vector.transpose(th[d][:], coord_view(ph, d))
            nc.vector.transpose(t2h[d][:], coord_view(p2h, d))
            nc.vector.transpose(tlo[d][:], coord_view(plob, d))
            nc.vector.transpose(t2lo[d][:], coord_view(p2lob, d))
        nc.vector.transpose(tsh[:], snh[:])
        nc.vector.transpose(tslo[:], snlob[:])

        # flatten DMAs (contiguous 128B/64B runs) into the row tiles;
        # xr/xl first so the main loop's matmuls can start ASAP (DMA queues
        # drain in issue order), the gather table after
        ones_b = stage.tile([2, N], bf16)
        nc.vector.memset(ones_b[:], 1.0)
        qs = (nc.sync, nc.scalar, nc.gpsimd)
        for d in range(3):
            qs[d].dma_start(xr[d:d + 1, :], t2h[d][:])
            qs[(d + 1) % 3].dma_start(xr[3 + d:4 + d, :], t2lo[d][:])
            qs[(d + 2) % 3].dma_start(xl[d:d + 1, :], th[d][:])
            qs[d].dma_start(xl[6 + d:7 + d, :], tlo[d][:])
        nc.sync.dma_start(xr[9:10, :], tsh[:])
        nc.scalar.dma_start(xr[10:11, :], tslo[:])
        nc.gpsimd.dma_start(xl[9:11, :], ones_b[:])
        # duplicated row groups via one contiguous copy each
        nc.scalar.dma_start(xl[3:6, :], xl[0:3, :])
        nc.sync.dma_start(xr[6:9, :], xr[0:3, :])
        nc.gpsimd.memset(tbl[:], 0.0)
        for d in range(3):
            qs[d].dma_start(tbl[d:d + 1, :], tp[d][:])

        # replicate coords to every 16-partition group of tbl
        engs = (nc.sync, nc.scalar, nc.gpsimd, nc.sync,
                nc.scalar, nc.gpsimd, nc.sync)
        for g in range(1, NG):
            engs[g - 1].dma_start(tbl[G16 * g:G16 * g + 3, :], tbl[0:3, :])

        # E[p, g] = 1 iff p//16 == g and p%16 < 3  (component-sum selector)
        ones3 = const.tile([3, 1], f32)
        nc.vector.memset(ones3[:], 1.0)
        esel = const.tile([P, NG], f32)
        nc.vector.memset(esel[:], 0.0)
        for j in range(NG):
            g = 2 * (j & 3) + (j >> 2)
            nc.sync.dma_start(esel[G16 * g:G16 * g + 3, j:j + 1], ones3[:])

        tr_sb = const.tile([G16, NG * NBLK], f32)
        # free layout of tr_sb: f = 64b + 32gl + 4rb + gh for row block
        # r = 8b + rb and group g = 2gh + gl  ->  DRAM block b is contiguous
        tr_view = tr_sb[:].rearrange("q (b gl rb gh) -> q b gl rb gh",
                                     b=4, gl=2, rb=8, gh=4)

        # ---- main loop over row blocks -------------------------------------
        for r in range(NBLK):
            lhsT = xl[:, r * P:(r + 1) * P]
            mval = mpool.tile([P, N], f32)
            for c in range(NCH):
                sl = slice(c * CH, (c + 1) * CH)
                ps = psum.tile([P, CH], f32, tag="mm")
                nc.tensor.matmul(ps[:], lhsT, xr[:, sl], start=True, stop=True)
                nc.scalar.copy(mval[:, sl], ps[:])

            v8 = small.tile([P, 8], f32, tag="v8")
            nc.vector.max(v8[:], mval[:])
            idx8 = small.tile([P, 8], dt.uint16, tag="idx8")
            nc.vector.max_index(idx8[:], v8[:], mval[:])

            # gather: group g gathers, for its 16 queries, slot-major:
            # gath[p, s*16+q16] = tbl[p, idx8[16*(p//16)+q16, s]]
            gath = small.tile([P, KNN * G16], f32, tag="gath")
            nc.gpsimd.indirect_copy(gath[:], tbl[:], idx8[:, :KNN], True)

            gv = gath[:].rearrange("p (s q) -> p q s", s=KNN, q=G16)
            ssum = small.tile([P, G16], f32, tag="ssum")
            nc.vector.tensor_reduce(ssum[:], gv, axis=Axis.X, op=Alu.add)
            mean = small.tile([P, G16], f32, tag="mean")
            nc.scalar.mul(mean[:], ssum[:], 1.0 / KNN)

            cent = small.tile([P, G16, KNN], f32, tag="cent")
            nc.gpsimd.tensor_sub(cent[:], gv,
                                 mean[:].unsqueeze(2).broadcast_to([P, G16, KNN]))
            nc.gpsimd.tensor_mul(cent[:], cent[:], cent[:])
            tt = small.tile([P, G16], f32, tag="tt")
            nc.vector.tensor_reduce(tt[:], cent[:], axis=Axis.X, op=Alu.add)

            ps_tr = psacc.tile([G16, NG], f32, tag="tr")
            nc.tensor.matmul(ps_tr[:], tt[:], esel[:], start=True, stop=True)
            nc.scalar.copy(tr_view[:, r // 8, :, r % 8, :],
                           ps_tr[:].rearrange("q (gl gh) -> q gl gh", gl=2))

        # ---- normalize + store ---------------------------------------------
        gmax = const.tile([G16, 1], f32)
        nc.vector.tensor_reduce(gmax[:], tr_sb[:], axis=Axis.X, op=Alu.max)
        gmax_all = const.tile([G16, 1], f32)
        nc.gpsimd.partition_all_reduce(gmax_all[:], gmax[:], channels=G16,
                                       reduce_op=bass_isa.ReduceOp.max)
        denom = const.tile([G16, 1], f32)
        nc.vector.tensor_scalar_add(denom[:], gmax_all[:], 1e-8)
        rec = const.tile([G16, 1], f32)
        nc.vector.reciprocal(rec[:], denom[:])
        outv = const.tile([G16, NG * NBLK], f32)
        nc.scalar.activation(outv[:], tr_sb[:],
                             mybir.ActivationFunctionType.Copy, scale=rec[:])

        # invert the permutation: query at wrapped slot (q16, r*8+g) with
        # r = 8b+rb, g = 2gh+gl is point n = 1024b + 512gl + 32q16 + 4rb + gh
        # per-b DMA: n = 1024b + 512gl + 32q + (4rb+gh); partition q must be
        # the outermost SBUF dim, innermost runs are 32 contiguous elements
        ov = outv[:].rearrange("q (b gl rbgh) -> b q gl rbgh",
                               b=4, gl=2, rbgh=32)
        od = out_d.ap().rearrange("(b gl q rbgh) -> b q gl rbgh",
                                  b=4, gl=2, q=G16, rbgh=32)
        qs2 = (nc.sync, nc.scalar, nc.gpsimd, nc.sync)
        for b in range(4):
            qs2[b].dma_start(od[b], ov[b])

    nc.compile()
    return nc


_NC_CACHE = {}


def kernel(pcd, k):
    pcd = np.asarray(pcd)
    k = int(np.asarray(k))
    assert k == KNN, f"kernel hardcodes k={KNN}, got {k}"
    B, n, d = pcd.shape
    assert (n, d) == (N, 3), f"kernel hardcodes N={N}, got {(n, d)}"

    from concourse.bass_utils import run_bass_kernel_spmd

    if "nc" not in _NC_CACHE:
        _NC_CACHE["nc"] = build_nc()
    nc = _NC_CACHE["nc"]

    in_maps = [{"pcd": np.ascontiguousarray(pcd[b], dtype=np.float32)}
               for b in range(B)]
    res = run_bass_kernel_spmd(nc, in_maps, list(range(B)))
    out = np.stack([res.results[b]["out"] for b in range(B)], axis=0)
    return out.astype(np.float32, copy=False)


if __name__ == "__main__":
    x = np.random.randn(8, N, 3).astype(np.float32)
    y = kernel(x, 5)
    print(y.shape, y.dtype, y[:2, :4])
